# revision 18
# baseline (speedup 1.0000x reference)
"""Block-diagonal projection kernel for Trainium2 (8 NeuronCores, SPMD).

Math: out[b,s,h,o] = sum_i inputs[b,s,h,i] * W[h,o,i]
Shapes: inputs [8, 2048, 16, 128] f32, W [16, 128, 128] f32.

Sharding: data-parallel over batch — core b handles inputs[b] (no
communication). Host-side layout prep:
  - puts the contraction dim (i) on SBUF partitions and pre-chunks the s
    axis so every input DMA reads 8 KB-contiguous per-partition lines;
  - splits x and W into bf16 hi/lo pairs (hi = bf16(v), lo = bf16(v - hi)).
    Per head the product is computed as three bf16 matmuls accumulated in
    fp32 PSUM: hi*hi + hi*lo + lo*hi (the dropped lo*lo term is ~2^-18
    relative). Same total HBM bytes as fp32 operands, but bf16 matmuls
    stream at 1 cycle/row vs 4 for fp32, cutting TensorE time ~25%.
  x per core: xhi/xlo [c, i=128, h=16, sc] bf16
  w (shared): whi/wlo [i=128, h=16, o=128] bf16
Per 128-row s-tile and head h: psum[s128, o] accumulates the three
terms with lhsT = x tile slice (stationary [i,128]) and rhs = w slice
([i, o=128]). Output lands in natural [s, h, o] fp32 layout — no
transposition anywhere on device.

Raw-bass engine programs (not Tile): walrus's PE instruction structs
accept at most one sync-wait per instruction, so all cross-engine sync
is standalone wait_ge instructions + then_inc updates:
  SP   : input DMAs, then a share of the trailing output tiles
  ACT  : w DMA, output DMAs (one per 128-row s-tile), trailing share
  PE   : 12 bf16 matmuls per (s-tile, head-group) into one PSUM bank
  DVE  : PSUM -> SBUF out-tile copies
DMA-completion waits use per-buffer-slot semaphores with total-count
thresholds (concurrent DMAs interleave their 16 per-engine increments,
so only fixed-set totals are race-free).
"""

from contextlib import ExitStack

import numpy as np

import concourse.bass as bass
import concourse.mybir as mybir
from concourse.bass_utils import run_bass_kernel_spmd

F32 = mybir.dt.float32
BF16 = mybir.dt.bfloat16

B, S, H, NI, NO = 8, 2048, 16, 128, 128
N_CORES = 8
SC = 128  # s rows per input chunk (hi+lo = 1 MiB per chunk)
XBUFS = 4  # x-chunk SBUF buffers
OBUFS = 3  # out-tile SBUF buffers
NBANKS = 8  # PSUM banks used (one head-group of 12 matmuls per bank)


def build_nc(s=S, h=H, ni=NI, no=NO, sc=SC):
    assert s % sc == 0 and sc == 128 and h % 4 == 0
    nt = s // 128  # 128-row s-tiles
    gpt = h // 4  # head-groups per s-tile
    ng = nt * gpt  # total matmul groups
    gpc = (sc // 128) * gpt  # groups per chunk
    ch = s // sc  # chunks

    nc = bass.Bass()
    xhi = nc.dram_tensor("xhi", [ch, ni, h, sc], BF16, kind="ExternalInput")
    xlo = nc.dram_tensor("xlo", [ch, ni, h, sc], BF16, kind="ExternalInput")
    whi = nc.dram_tensor("whi", [ni, h, no], BF16, kind="ExternalInput")
    wlo = nc.dram_tensor("wlo", [ni, h, no], BF16, kind="ExternalInput")
    y = nc.dram_tensor("y", [s, h, no], F32, kind="ExternalOutput")

    ctx = ExitStack()
    with ctx:
        xh_t = [ctx.enter_context(nc.sbuf_tensor(f"xh{i}", [ni, h, sc], BF16)) for i in range(XBUFS)]
        xl_t = [ctx.enter_context(nc.sbuf_tensor(f"xl{i}", [ni, h, sc], BF16)) for i in range(XBUFS)]
        ots = [ctx.enter_context(nc.sbuf_tensor(f"ot{i}", [128, h, no], F32)) for i in range(OBUFS)]
        wh_t = ctx.enter_context(nc.sbuf_tensor("wh", [ni, h, no], BF16))
        wl_t = ctx.enter_context(nc.sbuf_tensor("wl", [ni, h, no], BF16))
        pss = [ctx.enter_context(nc.psum_tensor(f"ps{i}", [128, 4, no], F32)) for i in range(NBANKS)]
        # Per-buffer-slot DMA-completion sems; waits are on fixed-set totals.
        s_x = [ctx.enter_context(nc.semaphore(f"s_x{i}")) for i in range(XBUFS)]
        s_yd = [ctx.enter_context(nc.semaphore(f"s_yd{i}")) for i in range(OBUFS)]
        # chunk 0 and w are split into per-head-group quarter DMAs so the
        # first matmuls start as soon as their slice lands.
        s_x0q = [ctx.enter_context(nc.semaphore(f"s_x0q{i}")) for i in range(gpt)]
        s_wq = [ctx.enter_context(nc.semaphore(f"s_wq{i}")) for i in range(gpt)]
        s_pe = ctx.enter_context(nc.semaphore("s_pe"))
        s_cp = ctx.enter_context(nc.semaphore("s_cp"))
        block = ctx.enter_context(nc.Block())

        def x_incs_through(c):
            # number of full-chunk fills (hi+lo pairs) on slot c%XBUFS up to
            # and including c
            return len([cc for cc in range(1, c + 1) if cc % XBUFS == c % XBUFS])

        # late output tiles alternate between the two HWDGE rings so both
        # flush the trailing backlog in parallel
        LATE = 4
        sp_tiles = [t for t in range(nt - LATE, nt - 1) if (t - nt) % 2 == 0]
        act_tiles = [t for t in range(nt - LATE, nt - 1) if (t - nt) % 2 == 1]

        def slot_incs_before(t):
            # full-DMA (16-inc) bundles landed on ot slot t%OBUFS before tile
            # t's copies may overwrite it
            return len([t2 for t2 in range(t - OBUFS + 1) if t2 % OBUFS == t % OBUFS])

        out_slot_total = [0] * OBUFS
        for t2 in range(nt - 1):
            out_slot_total[t2 % OBUFS] += 1
        out_slot_total[(nt - 1) % OBUFS] += gpt

        def emit_out_tile(eng, t):
            eng.wait_ge(s_cp, gpt * (t + 1))
            eng.dma_start(y[t * 128 : (t + 1) * 128, :, :], ots[t % OBUFS][:]).then_inc(
                s_yd[t % OBUFS], 16
            )

        def emit_last_tile_quarters(eng, qs):
            t = nt - 1
            for q in qs:
                eng.wait_ge(s_cp, gpt * t + q + 1)
                eng.dma_start(
                    y[t * 128 : (t + 1) * 128, 4 * q : 4 * (q + 1), :],
                    ots[t % OBUFS][:, 4 * q : 4 * (q + 1), :],
                ).then_inc(s_yd[t % OBUFS], 16)

        @block.sync
        def _(sp):
            for q in range(gpt):
                hs = slice(4 * q, 4 * (q + 1))
                sp.dma_start(xh_t[0][:, hs, :], xhi[0][:, hs, :]).then_inc(s_x0q[q], 16)
                sp.dma_start(xl_t[0][:, hs, :], xlo[0][:, hs, :]).then_inc(s_x0q[q], 16)
            for c in range(1, ch):
                if c >= XBUFS:
                    # buffer c%XBUFS free once chunk c-XBUFS fully consumed by PE
                    sp.wait_ge(s_pe, gpc * (c - XBUFS + 1))
                sp.dma_start(xh_t[c % XBUFS][:], xhi[c]).then_inc(s_x[c % XBUFS], 16)
                sp.dma_start(xl_t[c % XBUFS][:], xlo[c]).then_inc(s_x[c % XBUFS], 16)
            for t in sp_tiles:
                emit_out_tile(sp, t)
            emit_last_tile_quarters(sp, [0, 1])
            # data-landed barrier, split across the rings by final ownership
            sp.wait_ge(s_yd[(nt - 1) % OBUFS], 16 * out_slot_total[(nt - 1) % OBUFS])

        @block.tensor
        def _(pe):
            for g in range(ng):
                t = g // gpt  # s-tile index
                c = t * 128 // sc  # chunk index
                if t == 0:
                    pe.wait_ge(s_wq[g % gpt], 32)
                    pe.wait_ge(s_x0q[g % gpt], 32)
                elif g % gpc == 0:
                    pe.wait_ge(s_x[c % XBUFS], 32 * x_incs_through(c))
                if g >= NBANKS:
                    pe.wait_ge(s_cp, g - NBANKS + 1)
                xh = xh_t[c % XBUFS]
                xl = xl_t[c % XBUFS]
                t_in_c = t - c * (sc // 128)
                ss = slice(t_in_c * 128, (t_in_c + 1) * 128)
                ps = pss[g % NBANKS]
                for j in range(4):
                    hh = (g % gpt) * 4 + j
                    terms = (
                        (xh[:, hh, ss], wh_t[:, hh, :]),
                        (xh[:, hh, ss], wl_t[:, hh, :]),
                        (xl[:, hh, ss], wh_t[:, hh, :]),
                    )
                    for k, (lhsT, rhs) in enumerate(terms):
                        mm = pe.matmul(
                            ps[:, j, :],
                            lhsT,
                            rhs,
                            start=(j == 0 and k == 0),
                            stop=(j == 3 and k == 2),
                        )
                mm.then_inc(s_pe, 1)

        @block.vector
        def _(dve):
            for g in range(ng):
                t = g // gpt
                if t >= OBUFS and g % gpt == 0:
                    dve.wait_ge(s_yd[t % OBUFS], 16 * slot_incs_before(t))
                dve.wait_ge(s_pe, g + 1)
                gg = g % gpt
                dve.tensor_copy(
                    ots[t % OBUFS][:, gg * 4 : (gg + 1) * 4, :], pss[g % NBANKS][:]
                ).then_inc(s_cp, 1)

        @block.scalar
        def _(act):
            for q in range(gpt):
                hs = slice(4 * q, 4 * (q + 1))
                act.dma_start(wh_t[:, hs, :], whi[:, hs, :]).then_inc(s_wq[q], 16)
                act.dma_start(wl_t[:, hs, :], wlo[:, hs, :]).then_inc(s_wq[q], 16)
            for t in range(nt - LATE):
                emit_out_tile(act, t)
            for t in act_tiles:
                emit_out_tile(act, t)
            emit_last_tile_quarters(act, [2, 3])
            for i in range(OBUFS):
                if i != (nt - 1) % OBUFS:
                    act.wait_ge(s_yd[i], 16 * out_slot_total[i])

    return nc


_NC_CACHE = {}


def _get_nc():
    if "nc" not in _NC_CACHE:
        _NC_CACHE["nc"] = build_nc()
    return _NC_CACHE["nc"]


def _split_hi_lo(a):
    import ml_dtypes

    hi = a.astype(ml_dtypes.bfloat16)
    lo = (a - hi.astype(np.float32)).astype(ml_dtypes.bfloat16)
    return hi, lo


def run(inputs, W, trace=False):
    """Returns (out [B,S,H,NO] f32, BassKernelResults)."""
    import os

    if trace:
        os.environ.pop("BASS_NEVER_TRACE", None)
    else:
        # The axon NTFF profiling hook module isn't present in this image;
        # make sure a stray BASS_TRACE can't route us onto that path.
        os.environ.setdefault("BASS_NEVER_TRACE", "1")
    inputs = np.asarray(inputs, dtype=np.float32)
    W = np.asarray(W, dtype=np.float32)
    assert inputs.shape == (B, S, H, NI) and W.shape == (H, NO, NI)
    ch = S // SC
    # [b, s, h, i] -> [b, c, sc, h, i] -> [b, c, i, h, sc]
    xh = np.ascontiguousarray(inputs.reshape(B, ch, SC, H, NI).transpose(0, 1, 4, 3, 2))
    wh = np.ascontiguousarray(W.transpose(2, 0, 1))  # [i, h, o]
    xhi, xlo = _split_hi_lo(xh)
    whi, wlo = _split_hi_lo(wh)
    in_maps = [
        {"xhi": xhi[b], "xlo": xlo[b], "whi": whi, "wlo": wlo} for b in range(N_CORES)
    ]
    br = run_bass_kernel_spmd(_get_nc(), in_maps, list(range(N_CORES)), trace=trace)
    out = np.stack([r["y"] for r in br.results])  # [b, s, h, o]
    return out, br


def kernel(inputs, W):
    out, _ = run(inputs, W)
    return out
